# revision 21
# baseline (speedup 1.0000x reference)
"""Trainium2 Bass kernel for nn_Dilate: 7x7 all-ones conv (same padding) -> (y > 0) int32 mask.

Input  x: (16, 1, 1024, 1024) float32, weight: (1, 1, 7, 7) ones (values unused).
Output:   (16, 1, 1024, 1024) int32 in {0, 1}.

Per core (pure batch data-parallel, 2 images/core on 8 cores), per 128-row tile:
  - The separable 7x7 box-sum is computed with the horizontal 7-tap stage
    SPLIT across two engines so neither is the lone bottleneck:
      * scan tiles: a DVE tensor_tensor_scan over the padded fp16 input rows
        produces the horizontal sliding sums X7 (fp32 state, fp16 out,
        recurrence-bound at ~2.1 cyc/elem); one banded fp16 matmul
        (ones-band lhsT [128,122]) then adds the 7 vertical taps.
      * PE tiles: the horizontal stage is folded into the matmul itself as
        7 accumulating matmuls whose rhs are column-shifted slices of the
        same padded input tile (boxsum2d = sum_dx bandV^T x[:, j+dx]) - no
        DVE work at all, and bit-better precision (no fp16 X7 rounding).
  - ACT sigmoid(1e8*D) + round-to-int8 thresholds PSUM->SBUF in one pass
    (decision boundary exactly at D=0) into a per-image accumulation buffer.
  - Masks leave in 3 batched rearranged HWDGE stores per image (scalar
    ring); per-tile SWDGE stores measured only ~54 GB/s and left a >10us
    drain tail.
  - x rows are padded host-side with 7+3 zero columns so loads are
    full-tile pool writes (pool slot reuse carries the WAR deps; DMA into
    slices of manually rotated tiles does not - that cost a debugging
    session) and the zero pads give the horizontal stage its edge handling.
  - Precision: fp16 input quantization + fp16 X7 rounding give rel_err
    ~0.013 on the 0/1 mask (gate 2e-2); verified in numpy ahead of time.
"""

import numpy as np

import concourse.bacc as bacc
import concourse.mybir as mybir
from concourse.tile import TileContext
from concourse.bass_utils import run_bass_kernel_spmd

B, H, W = 16, 1024, 1024
NCORES = 8
PER_CORE = B // NCORES  # 2 images per core
R = 7
PAD = R // 2  # 3
P = 128             # SBUF partitions per tile (input rows incl. halo)
MOUT = P - (R - 1)  # 122 output rows per tile
NTILES = -(-H // MOUT)  # 9 row tiles per image
XBW = R + W + PAD   # padded x row: 7 leading + 3 trailing zero cols
SFD = W + PAD       # scan free dim: 1027 (output j's box-sum lands at col j+3)

SIG_SCALE = 1.0e8   # pre-scale for the sigmoid threshold trick
N_XB = 18           # one input buffer per tile: zero recycling, so no load
                    # is ever gated on a PE-path tile's late buffer release
PE_TILES = {2, 5, 8, 11, 14}  # tiles whose horizontal stage runs on the PE


def _band_matrices() -> np.ndarray:
    """bands[0]: t=0 (partition p = image row p, top clamp);
    bands[1]: interior (partition p = row o0-3+p);
    bands[2]: last tile (partition p = row H-128+p, bottom clamp).
    band[k, m] = 1 iff output row m sums input partition k."""
    bands = np.zeros((3, P, MOUT), dtype=np.float32)
    for m in range(MOUT):
        bands[0, max(0, m - PAD) : m + PAD + 1, m] = 1.0
        bands[1, m : m + R, m] = 1.0
    # last tile: outputs start at row H-48 = partition 80
    for m in range(48):
        bands[2, 80 + m - PAD : min(80 + m + PAD + 1, P), m] = 1.0
    return bands


def _build_program():
    nc = bacc.Bacc("TRN2")
    f16 = mybir.dt.float16
    x_d = nc.dram_tensor("x", [PER_CORE, H, XBW], f16, kind="ExternalInput")
    band_d = nc.dram_tensor("band", [3, P, MOUT], f16, kind="ExternalInput")
    # y is partition-major (p, tile, w) so each partition's batched store is
    # a 4KB-contiguous HBM chunk (the natural row layout gives 1KB scattered
    # chunks, which drain at only ~55 GB/s on every DGE path); the host
    # transposes back to image rows.
    y_d = nc.dram_tensor(
        "y", [PER_CORE, MOUT, NTILES, W], mybir.dt.int8, kind="ExternalOutput")

    add = mybir.AluOpType.add
    sub = mybir.AluOpType.subtract
    sig = mybir.ActivationFunctionType.Sigmoid

    with TileContext(nc) as tc:
        with (
            tc.tile_pool(name="const", bufs=1) as cpool,
            tc.tile_pool(name="xin", bufs=N_XB) as xbpool,
            tc.tile_pool(name="x7", bufs=13) as x7pool,
            tc.tile_pool(name="psum", bufs=4, space="PSUM") as psum_pool,
        ):
            band_ts = []
            for i in range(3):
                bt = cpool.tile([P, MOUT], f16, tag=f"band{i}")
                nc.scalar.dma_start(out=bt[:], in_=band_d[i])
                band_ts.append(bt)

            # per-image mask accumulator; every slot is written exactly once
            # so the batched stores have no WAR hazards.
            acc = cpool.tile([P, PER_CORE, NTILES, W], mybir.dt.int8, tag="acc")

            tiles = []  # (band_idx, img, row_lo, tile_idx_in_image)
            for img in range(PER_CORE):
                for t in range(NTILES):
                    o0 = t * MOUT
                    if t == 0:
                        lo = 0
                    elif t == NTILES - 1:
                        lo = H - P
                    else:
                        lo = o0 - PAD
                    tiles.append(
                        (0 if t == 0 else (2 if t == NTILES - 1 else 1),
                         img, lo, t))

            # Pre-emit every input load (highest scheduler priority ->
            # depth-N_XB prefetch on the otherwise idle sync HWDGE ring).
            x_tiles = []
            for bi, img, lo, t in tiles:
                xt = xbpool.tile([P, XBW], f16)
                nc.sync.dma_start(out=xt[:], in_=x_d[img, lo : lo + P, :])
                x_tiles.append(xt)

            def emit_store(img, t0, nt):
                # SWDGE on the idle gpsimd queue: HWDGE stores occupy the
                # issuing engine queue for the whole drain and starve it.
                # All store paths drain at only ~75-85 GB/s, so start each
                # tile's store as early as possible.
                nc.gpsimd.dma_start(
                    out=y_d[img, :, t0 : t0 + nt, :],
                    in_=acc[0:MOUT, img, t0 : t0 + nt, :])

            # All scans first: program order is scheduler priority, and the
            # DVE is the critical engine - this makes its planned stream
            # back-to-back scans with no cross-engine waits (the scheduler's
            # cost model underestimates scans ~2x and otherwise trips
            # conservative waits near the tail).  Each x7 buffer is used
            # once (bufs = #scan tiles), so no recycling deps either.
            x7s = {}
            for i, (bi, img, lo, t) in enumerate(tiles):
                if i in PE_TILES:
                    continue
                x7 = x7pool.tile([P, SFD], f16)
                # X7[:, c] = sum of x cols c-6..c (padded) = boxsum(j=c-3)
                nc.vector.tensor_tensor_scan(
                    x7[:], x_tiles[i][:, R : R + SFD], x_tiles[i][:, 0:SFD],
                    0.0, add, sub)
                x7s[i] = x7

            # Per-tile matmuls + threshold + store trail the scans.
            for i, (bi, img, lo, t) in enumerate(tiles):
                xt = x_tiles[i]
                bt = band_ts[bi]
                d_ps = psum_pool.tile([MOUT, W], mybir.dt.float32)

                if i in PE_TILES:
                    # horizontal+vertical fully on PE: 7 accumulating
                    # matmuls per 512-col half, rhs shifted by dx.
                    for j in range(2):
                        for dxi in range(R):
                            nc.tensor.matmul(
                                d_ps[:, j * 512 : (j + 1) * 512],
                                bt[:],
                                xt[:, dxi + 4 + j * 512 : dxi + 4 + j * 512 + 512],
                                start=(dxi == 0), stop=(dxi == R - 1),
                            )
                else:
                    for j in range(2):
                        nc.tensor.matmul(
                            d_ps[:, j * 512 : (j + 1) * 512],
                            bt[:],
                            x7s[i][:, PAD + j * 512 : PAD + (j + 1) * 512],
                            start=True, stop=True,
                        )

                # threshold: mask = D > 0 -> int8, one ACT pass from PSUM
                nc.scalar.activation(
                    acc[0:MOUT, img, t, :], d_ps[:], sig, scale=SIG_SCALE)

                # store each tile as soon as it is thresholded (contiguous
                # per-partition chunks on both sides)
                emit_store(img, t, 1)

    nc.compile()
    return nc


_PROGRAM_CACHE = {}


def _get_program():
    if "nc" not in _PROGRAM_CACHE:
        _PROGRAM_CACHE["nc"] = _build_program()
    return _PROGRAM_CACHE["nc"]


def _make_in_maps(xs):
    """xs: (B, H, W) float array -> per-core input maps (fp16, row-padded)."""
    xs16 = np.zeros((B, H, XBW), dtype=np.float16)
    xs16[:, :, R : R + W] = np.asarray(xs).reshape(B, H, W)
    band = _band_matrices().astype(np.float16)
    return [
        {"x": np.ascontiguousarray(xs16[c * PER_CORE : (c + 1) * PER_CORE]),
         "band": band}
        for c in range(NCORES)
    ]


def kernel(x, weight=None, **_unused):
    x = np.asarray(x)
    assert x.shape == (B, 1, H, W), x.shape
    nc = _get_program()
    in_maps = _make_in_maps(x.reshape(B, H, W))
    res = run_bass_kernel_spmd(nc, in_maps, core_ids=list(range(NCORES)))
    ys = np.concatenate([r["y"] for r in res.results], axis=0)  # (B, MOUT, NTILES, W)
    out = np.empty((B, H, W), dtype=np.int8)
    full = (NTILES - 1) * MOUT  # 976 rows from the 8 full tiles
    out[:, :full, :] = (
        ys[:, :, : NTILES - 1, :].transpose(0, 2, 1, 3).reshape(B, full, W))
    out[:, full:, :] = ys[:, : H - full, NTILES - 1, :]
    return out.reshape(B, 1, H, W).astype(np.int32)
